# revision 5
# baseline (speedup 1.0000x reference)
"""GRU layer kernel for 8 Trainium2 NeuronCores.

Sharding: data-parallel over batch B=64 -> 8 cores x B_local=8 (no
cross-core communication; weights replicated, per-core batch slice).

Layout: everything transposed on-chip ([128 j-partitions, ...]):
 - Phase 1: gates g = x @ W_x.T + b precomputed for the whole sequence
   with fp32r matmuls (W stationary, xT streaming), biases folded into
   the PSUM->SBUF evict, stored bf16 in DRAM as [24 m][128 j][8 b][512 t].
 - Phase 2: 512-step scan. Recurrent matmuls W-stationary bf16 (FWL),
   h kept fp32 and split hi/lo bf16 per step for near-fp32 precision.
   Gate adds via identity matmul into the same PSUM accumulation.
"""
import sys
if '/opt/trn_rl_repo' not in sys.path:
    sys.path.insert(0, '/opt/trn_rl_repo')

import numpy as np
import ml_dtypes

import concourse.bass as bass
import concourse.mybir as mybir
import concourse.tile as tile
from concourse.bass_utils import run_bass_kernel_spmd

# ---------------------------------------------------------------- BIR patch
# This walrus build rejects >1 inline sync wait per instruction; hoist extras
# onto same-engine NoOps inserted immediately before (identical semantics).
import json as _json


def _split_multi_waits(bir_json: bytes) -> bytes:
    d = _json.loads(bir_json)
    uid = [0]

    def mk_nop(engine, wait, debug):
        uid[0] += 1
        return {
            "debug": debug, "engine": engine, "ins": [],
            "name": f"I-waitsplit-{uid[0]}", "opcode": "NoOp", "outs": [],
            "sync_info": {"on_wait": [wait], "on_update": []},
            "text_hint": "waitsplit",
        }

    for f in d["functions"]:
        for blk in f["blocks"]:
            out = []
            for inst in blk["instructions"]:
                si = inst.get("sync_info")
                ow = (si or {}).get("on_wait") or []
                if len(ow) > 1:
                    for w in ow[:-1]:
                        out.append(mk_nop(inst["engine"], w, inst.get("debug", 0)))
                    si["on_wait"] = [ow[-1]]
                out.append(inst)
            blk["instructions"] = out
    return _json.dumps(d).encode()


_PATCHED = False


def _install_patch():
    global _PATCHED
    if _PATCHED:
        return
    import concourse.bass_utils as bu
    import concourse.bass2jax as b2j
    orig = bu.compile_bir_kernel

    def patched(bir_json, tmpdir, neff_name="file.neff"):
        if isinstance(bir_json, str):
            bir_json = bir_json.encode()
        return orig(_split_multi_waits(bir_json), tmpdir, neff_name)

    bu.compile_bir_kernel = patched
    b2j.compile_bir_kernel = patched
    _PATCHED = True


# ---------------------------------------------------------------- constants
T, B, I, H = 512, 64, 1024, 1024
NCORES = 8
BL = B // NCORES            # 8 batch rows per core
P = 128
KT = I // P                 # 8 contraction tiles
MT_RZ = 2 * H // P          # 16 m-tiles for r,z
MT_H = H // P               # 8 m-tiles for candidate
MT_ALL = 3 * H // P         # 24 m-tiles in phase 1
TBLK = 64                   # phase-1 t-block / phase-2 gate chunk (steps)
NBLK = T // TBLK            # 8 blocks

F32 = mybir.dt.float32
F32R = mybir.dt.float32r
BF16 = mybir.dt.bfloat16

_BUILD_CACHE = {}


def build(t_steps=T):
    nc = bass.Bass(trn_type="TRN2")
    nblk = (t_steps + TBLK - 1) // TBLK

    # ---- DRAM parameters (per-core shapes) ----
    # xT: [128, KT, BL*t_steps] fp32r  (partition=k within tile, k-tile, cols)
    # column order: (b, t) t-fastest within block? -> see host prep: (blk, b, t)
    xT = nc.dram_tensor("xT", [P, KT, BL * t_steps], F32R, kind="ExternalInput")
    # phase-1 weights (stationary): [128, KT, 3H] fp32r ; row j = m output
    wx = nc.dram_tensor("wx", [P, KT, 3 * H], F32R, kind="ExternalInput")
    # biases for the 24 m-tiles: [128, 24] fp32 (per-partition scalars)
    bias = nc.dram_tensor("bias", [P, MT_ALL], F32, kind="ExternalInput")
    # phase-2 weights bf16: [128, KT, 2H] and [128, KT, H]
    wrz = nc.dram_tensor("wrz", [P, KT, 2 * H], BF16, kind="ExternalInput")
    whh = nc.dram_tensor("whh", [P, KT, H], BF16, kind="ExternalInput")
    # identity 128x128 bf16
    ident = nc.dram_tensor("ident", [P, P], BF16, kind="ExternalInput")
    # initial state transposed: [128, KT, BL] fp32
    h0 = nc.dram_tensor("h0", [P, KT, BL], F32, kind="ExternalInput")
    # outputs
    # native layout [t, p, k, b]; host transposes to [t, b, H]
    ys = nc.dram_tensor("ys", [t_steps, P, KT, BL], F32, kind="ExternalOutput")
    hT_out = nc.dram_tensor("hT_out", [P, KT, BL], F32, kind="ExternalOutput")
    # intermediate gates in DRAM, bf16: [MT_ALL, 128, BL, t_steps]
    gates = nc.dram_tensor("gates", [MT_ALL, P, BL, t_steps], BF16)

    with tile.TileContext(nc) as tc:
        # ================= PHASE 1: gate projections =================
        with tc.tile_pool(name="p1w", bufs=1) as p1w, \
             tc.tile_pool(name="p1x", bufs=3) as p1x, \
             tc.tile_pool(name="p1o", bufs=4) as p1o, \
             tc.tile_pool(name="p1b", bufs=1) as p1b, \
             tc.tile_pool(name="p1ps", bufs=4, space="PSUM") as p1ps:
            # resident weights + biases
            wt = p1w.tile([P, KT, 3 * H], F32R)
            nc.sync.dma_start(wt[:], wx[:])
            bt = p1b.tile([P, MT_ALL], F32)
            nc.sync.dma_start(bt[:], bias[:])

            ncols = BL * TBLK  # 512 columns per t-block
            for blk in range(nblk):
                cols = slice(blk * ncols, (blk + 1) * ncols)
                xt_ = p1x.tile([P, KT, ncols], F32R, tag="xtile")
                nc.sync.dma_start(xt_[:], xT[:, :, cols])
                for m in range(MT_ALL):
                    ps = p1ps.tile([P, ncols], F32, tag="ps")
                    for k in range(KT):
                        nc.tensor.matmul(
                            ps[:], wt[:, k, m * P:(m + 1) * P], xt_[:, k, :],
                            start=(k == 0), stop=(k == KT - 1))
                    ot = p1o.tile([P, BL, TBLK], BF16, tag="gout")
                    # psum -> sbuf bf16 with bias folded in
                    nc.scalar.activation(
                        ot[:], ps[:].rearrange("p (b t) -> p b t", b=BL),
                        mybir.ActivationFunctionType.Identity,
                        bias=bt[:, m:m + 1])
                    nc.sync.dma_start(
                        gates[m, :, :, blk * TBLK:(blk + 1) * TBLK], ot[:])

        # ================= PHASE 2: recurrent scan =================
        with tc.tile_pool(name="p2w", bufs=1) as p2w, \
             tc.tile_pool(name="p2g", bufs=2) as p2g, \
             tc.tile_pool(name="p2h", bufs=3) as p2h, \
             tc.tile_pool(name="p2t", bufs=3) as p2t, \
             tc.tile_pool(name="p2y", bufs=3) as p2y, \
             tc.tile_pool(name="p2ps", bufs=2, space="PSUM") as p2ps:
            wrz_t = p2w.tile([P, KT, 2 * H], BF16)
            nc.sync.dma_start(wrz_t[:], wrz[:])
            whh_t = p2w.tile([P, KT, H], BF16)
            nc.sync.dma_start(whh_t[:], whh[:])
            id_t = p2w.tile([P, P], BF16)
            nc.sync.dma_start(id_t[:], ident[:])

            hT = p2h.tile([P, KT, BL], F32, tag="hT")
            nc.sync.dma_start(hT[:], h0[:])

            for blk in range(nblk):
                gc = p2g.tile([P, MT_ALL, BL, TBLK], BF16, tag="gchunk")
                nc.sync.dma_start(
                    gc[:], gates[:, :, :, blk * TBLK:(blk + 1) * TBLK]
                    .rearrange("m p b t -> p m b t"))
                for tt in range(TBLK):
                    t = blk * TBLK + tt
                    if t >= t_steps:
                        break
                    # ---- h -> hi/lo bf16 ----
                    h_hi = p2t.tile([P, KT, BL], BF16, tag="h_hi")
                    nc.vector.tensor_copy(h_hi[:], hT[:])
                    h_rem = p2t.tile([P, KT, BL], F32, tag="h_rem")
                    nc.vector.tensor_sub(h_rem[:], hT[:], h_hi[:])
                    h_lo = p2t.tile([P, KT, BL], BF16, tag="h_lo")
                    nc.vector.tensor_copy(h_lo[:], h_rem[:])

                    # ---- r,z matmuls ----
                    ps_rz = p2ps.tile([P, MT_RZ, BL], F32, tag="ps_rz")
                    # gate add via identity matmul (one ldweights, 1 mm)
                    nc.tensor.matmul(
                        ps_rz[:].rearrange("p m b -> p (m b)"),
                        id_t[:],
                        gc[:, 0:MT_RZ, :, tt].rearrange("p m b -> p (m b)"),
                        start=True, stop=False, skip_group_check=True)
                    for m in range(MT_RZ):
                        for k in range(KT):
                            w_ap = wrz_t[:, k, m * P:(m + 1) * P]
                            nc.tensor.matmul(
                                ps_rz[:, m, :], w_ap, h_hi[:, k, :],
                                start=False, stop=False,
                                skip_group_check=True)
                            nc.tensor.matmul(
                                ps_rz[:, m, :], w_ap, h_lo[:, k, :],
                                start=False,
                                stop=(m == MT_RZ - 1 and k == KT - 1),
                                skip_group_check=True)
                    rz = p2t.tile([P, MT_RZ, BL], F32, tag="rz")
                    nc.scalar.activation(
                        rz[:], ps_rz[:], mybir.ActivationFunctionType.Sigmoid)

                    # ---- m = r * h (hi/lo) ----
                    mm_f = p2t.tile([P, KT, BL], F32, tag="mm_f")
                    nc.vector.tensor_mul(mm_f[:], rz[:, 0:KT, :], hT[:])
                    m_hi = p2t.tile([P, KT, BL], BF16, tag="m_hi")
                    nc.vector.tensor_copy(m_hi[:], mm_f[:])
                    m_rem = p2t.tile([P, KT, BL], F32, tag="m_rem")
                    nc.vector.tensor_sub(m_rem[:], mm_f[:], m_hi[:])
                    m_lo = p2t.tile([P, KT, BL], BF16, tag="m_lo")
                    nc.vector.tensor_copy(m_lo[:], m_rem[:])

                    # ---- candidate matmuls ----
                    ps_c = p2ps.tile([P, MT_H, BL], F32, tag="ps_c")
                    nc.tensor.matmul(
                        ps_c[:].rearrange("p m b -> p (m b)"),
                        id_t[:],
                        gc[:, MT_RZ:MT_ALL, :, tt].rearrange("p m b -> p (m b)"),
                        start=True, stop=False, skip_group_check=True)
                    for m in range(MT_H):
                        for k in range(KT):
                            w_ap = whh_t[:, k, m * P:(m + 1) * P]
                            nc.tensor.matmul(
                                ps_c[:, m, :], w_ap, m_hi[:, k, :],
                                start=False, stop=False,
                                skip_group_check=True)
                            nc.tensor.matmul(
                                ps_c[:, m, :], w_ap, m_lo[:, k, :],
                                start=False,
                                stop=(m == MT_H - 1 and k == KT - 1),
                                skip_group_check=True)
                    cT = p2t.tile([P, MT_H, BL], F32, tag="cT")
                    nc.scalar.activation(
                        cT[:], ps_c[:], mybir.ActivationFunctionType.Tanh)

                    # ---- blend: h' = c + z*(h - c) ----
                    z_ap = rz[:, KT:MT_RZ, :]
                    hmc = p2t.tile([P, KT, BL], F32, tag="hmc")
                    nc.vector.tensor_sub(hmc[:], hT[:], cT[:])
                    zh = p2t.tile([P, KT, BL], F32, tag="zh")
                    nc.vector.tensor_mul(zh[:], z_ap, hmc[:])
                    hT = p2h.tile([P, KT, BL], F32, tag="hT")
                    nc.vector.tensor_add(hT[:], cT[:], zh[:])

                    # ---- ys write (contiguous DMA, off critical path) ----
                    nc.sync.dma_start(ys[t], hT[:])

            nc.sync.dma_start(hT_out[:], hT[:])
    return nc


# ---------------------------------------------------------------- host side
def _prep_in_maps(x, state, w_xr_w, w_xr_b, w_hr_w, w_xz_w, w_xz_b, w_hz_w,
                  w_xh_w, w_xh_b, w_hh_w, t_steps=T):
    x = np.asarray(x)[:t_steps]
    state = np.asarray(state)

    # phase-1 stationary weights: w rows j -> lhsT [k, j]; [128, KT, 3H]
    wx_all = np.concatenate([w_xr_w, w_xz_w, w_xh_w], axis=0)  # [3H, I]
    wxT = np.ascontiguousarray(wx_all.T)                       # [I, 3H]
    wx_t = np.ascontiguousarray(
        wxT.reshape(KT, P, 3 * H).transpose(1, 0, 2)).astype(np.float32)

    bias_all = np.concatenate([w_xr_b, w_xz_b, w_xh_b])        # [3H]
    bias_t = np.ascontiguousarray(
        bias_all.reshape(MT_ALL, P).T).astype(np.float32)      # [128, 24]

    wrz_all = np.concatenate([w_hr_w, w_hz_w], axis=0)         # [2H, H]
    wrz_t = np.ascontiguousarray(
        wrz_all.T.reshape(KT, P, 2 * H).transpose(1, 0, 2)
    ).astype(ml_dtypes.bfloat16)
    whh_t = np.ascontiguousarray(
        w_hh_w.T.reshape(KT, P, H).transpose(1, 0, 2)
    ).astype(ml_dtypes.bfloat16)

    ident = np.eye(P, dtype=np.float32).astype(ml_dtypes.bfloat16)

    in_maps = []
    for c in range(NCORES):
        bs = slice(c * BL, (c + 1) * BL)
        xc = x[:, bs, :]                                       # [t, BL, I]
        # columns ordered (blk, b, t_within_blk)
        nblk = (t_steps + TBLK - 1) // TBLK
        xc_b = xc.reshape(nblk, TBLK, BL, I)
        # -> [I, blk, b, t] -> [P, KT, blk*b*t]
        xcT = np.ascontiguousarray(xc_b.transpose(3, 0, 2, 1))  # [I, blk, b, t]
        xT_c = np.ascontiguousarray(
            xcT.reshape(KT, P, nblk * BL * TBLK).transpose(1, 0, 2)
        ).astype(np.float32)

        h0_c = np.ascontiguousarray(
            state[bs, :].T.reshape(KT, P, BL).transpose(1, 0, 2)
        ).astype(np.float32)

        in_maps.append({
            "xT": xT_c, "wx": wx_t, "bias": bias_t,
            "wrz": wrz_t, "whh": whh_t, "ident": ident,
            "h0": h0_c,
        })
    return in_maps


def _assemble(results, t_steps=T):
    ys_parts = []
    hf_parts = []
    for c in range(NCORES):
        y = results[c]["ys"]                                   # [t, P, KT, BL]
        ys_parts.append(
            np.ascontiguousarray(y.transpose(0, 3, 2, 1)).reshape(t_steps, BL, H))
        hT_c = results[c]["hT_out"]                            # [P, KT, BL]
        hf_parts.append(hT_c.transpose(2, 1, 0).reshape(BL, H))
    ys = np.concatenate(ys_parts, axis=1)                      # [t, B, H]
    final = np.concatenate(hf_parts, axis=0)                   # [B, H]
    return ys, final


def kernel(x, state, w_xr_w, w_xr_b, w_hr_w, w_xz_w, w_xz_b, w_hz_w,
           w_xh_w, w_xh_b, w_hh_w, t_steps=T):
    _install_patch()
    key = t_steps
    if key not in _BUILD_CACHE:
        _BUILD_CACHE[key] = build(t_steps)
    nc = _BUILD_CACHE[key]
    in_maps = _prep_in_maps(x, state, w_xr_w, w_xr_b, w_hr_w, w_xz_w,
                            w_xz_b, w_hz_w, w_xh_w, w_xh_b, w_hh_w, t_steps)
    res = run_bass_kernel_spmd(nc, in_maps, core_ids=list(range(NCORES)))
    return _assemble(res.results, t_steps)


# revision 6
# speedup vs baseline: 103.8866x; 103.8866x over previous
"""GRU layer kernel for 8 Trainium2 NeuronCores.

Sharding: data-parallel over batch B=64 -> 8 cores x B_local=8 (no
cross-core communication; weights replicated, per-core batch slice).

Layout: everything transposed on-chip ([128 j-partitions, ...]):
 - Phase 1: gates g = x @ W_x.T + b precomputed for the whole sequence
   with fp32r matmuls (W stationary, xT streaming), biases folded into
   the PSUM->SBUF evict, stored bf16 in DRAM as [24 m][128 j][8 b][512 t].
 - Phase 2: 512-step scan. Recurrent matmuls W-stationary bf16 (FWL),
   h kept fp32 and split hi/lo bf16 per step for near-fp32 precision.
   Gate adds via identity matmul into the same PSUM accumulation.
"""
import sys
if '/opt/trn_rl_repo' not in sys.path:
    sys.path.insert(0, '/opt/trn_rl_repo')

import numpy as np
import ml_dtypes

import concourse.bass as bass
import concourse.mybir as mybir
import concourse.tile as tile
from concourse.bass_utils import run_bass_kernel_spmd

# ---------------------------------------------------------------- BIR patch
# This walrus build rejects >1 inline sync wait per instruction; hoist extras
# onto same-engine NoOps inserted immediately before (identical semantics).
import json as _json


def _split_multi_waits(bir_json: bytes) -> bytes:
    d = _json.loads(bir_json)
    uid = [0]

    def mk_nop(engine, wait, debug):
        uid[0] += 1
        return {
            "debug": debug, "engine": engine, "ins": [],
            "name": f"I-waitsplit-{uid[0]}", "opcode": "NoOp", "outs": [],
            "sync_info": {"on_wait": [wait], "on_update": []},
            "text_hint": "waitsplit",
        }

    for f in d["functions"]:
        for blk in f["blocks"]:
            out = []
            for inst in blk["instructions"]:
                si = inst.get("sync_info")
                ow = (si or {}).get("on_wait") or []
                if len(ow) > 1:
                    for w in ow[:-1]:
                        out.append(mk_nop(inst["engine"], w, inst.get("debug", 0)))
                    si["on_wait"] = [ow[-1]]
                out.append(inst)
            blk["instructions"] = out
    return _json.dumps(d).encode()


_PATCHED = False


def _install_patch():
    global _PATCHED
    if _PATCHED:
        return
    import concourse.bass_utils as bu
    import concourse.bass2jax as b2j
    orig = bu.compile_bir_kernel

    def patched(bir_json, tmpdir, neff_name="file.neff"):
        if isinstance(bir_json, str):
            bir_json = bir_json.encode()
        return orig(_split_multi_waits(bir_json), tmpdir, neff_name)

    bu.compile_bir_kernel = patched
    b2j.compile_bir_kernel = patched
    _PATCHED = True


# ---------------------------------------------------------------- constants
T, B, I, H = 512, 64, 1024, 1024
NCORES = 8
BL = B // NCORES            # 8 batch rows per core
P = 128
KT = I // P                 # 8 contraction tiles
MT_RZ = 2 * H // P          # 16 m-tiles for r,z
MT_H = H // P               # 8 m-tiles for candidate
MT_ALL = 3 * H // P         # 24 m-tiles in phase 1
TBLK = 64                   # phase-1 t-block / phase-2 gate chunk (steps)
NBLK = T // TBLK            # 8 blocks

F32 = mybir.dt.float32
F32R = mybir.dt.float32r
BF16 = mybir.dt.bfloat16

_BUILD_CACHE = {}


def build(t_steps=T):
    nc = bass.Bass(trn_type="TRN2")
    nblk = (t_steps + TBLK - 1) // TBLK

    # ---- DRAM parameters (per-core shapes) ----
    # xT: [128, KT, BL*t_steps] fp32r  (partition=k within tile, k-tile, cols)
    # column order: (b, t) t-fastest within block? -> see host prep: (blk, b, t)
    xT = nc.dram_tensor("xT", [P, KT, BL * t_steps], F32R, kind="ExternalInput")
    # phase-1 weights (stationary): [128, KT, 3H] fp32r ; row j = m output
    wx = nc.dram_tensor("wx", [P, KT, 3 * H], F32R, kind="ExternalInput")
    # biases for the 24 m-tiles: [128, 24] fp32 (per-partition scalars)
    bias = nc.dram_tensor("bias", [P, MT_ALL], F32, kind="ExternalInput")
    # phase-2 weights bf16: [128, KT, 2H] and [128, KT, H]
    wrz = nc.dram_tensor("wrz", [P, KT, 2 * H], BF16, kind="ExternalInput")
    whh = nc.dram_tensor("whh", [P, KT, H], BF16, kind="ExternalInput")
    # identity 128x128 bf16
    ident = nc.dram_tensor("ident", [P, P], BF16, kind="ExternalInput")
    # initial state transposed: [128, KT, BL] fp32
    h0 = nc.dram_tensor("h0", [P, KT, BL], F32, kind="ExternalInput")
    # outputs
    # native layout [t, p, k, b]; host transposes to [t, b, H]
    ys = nc.dram_tensor("ys", [t_steps, P, KT, BL], F32, kind="ExternalOutput")
    hT_out = nc.dram_tensor("hT_out", [P, KT, BL], F32, kind="ExternalOutput")
    # intermediate gates in DRAM, bf16: [MT_ALL, 128, BL, t_steps]
    gates = nc.dram_tensor("gates", [nblk, MT_ALL, P, BL, TBLK], BF16)

    with tile.TileContext(nc) as tc:
        # ================= PHASE 1: gate projections =================
        with tc.tile_pool(name="p1w", bufs=1) as p1w, \
             tc.tile_pool(name="p1x", bufs=3) as p1x, \
             tc.tile_pool(name="p1o", bufs=4) as p1o, \
             tc.tile_pool(name="p1b", bufs=1) as p1b, \
             tc.tile_pool(name="p1ps", bufs=4, space="PSUM") as p1ps:
            # resident weights + biases
            wt = p1w.tile([P, KT, 3 * H], F32R)
            nc.sync.dma_start(wt[:], wx[:])
            bt = p1b.tile([P, MT_ALL], F32)
            nc.sync.dma_start(bt[:], bias[:])

            ncols = BL * TBLK  # 512 columns per t-block
            for blk in range(nblk):
                cols = slice(blk * ncols, (blk + 1) * ncols)
                xt_ = p1x.tile([P, KT, ncols], F32R, tag="xtile")
                nc.sync.dma_start(xt_[:], xT[:, :, cols])
                for m in range(MT_ALL):
                    ps = p1ps.tile([P, ncols], F32, tag="ps")
                    for k in range(KT):
                        nc.tensor.matmul(
                            ps[:], wt[:, k, m * P:(m + 1) * P], xt_[:, k, :],
                            start=(k == 0), stop=(k == KT - 1))
                    ot = p1o.tile([P, BL, TBLK], BF16, tag="gout")
                    # psum -> sbuf bf16 with bias folded in
                    nc.scalar.activation(
                        ot[:], ps[:].rearrange("p (b t) -> p b t", b=BL),
                        mybir.ActivationFunctionType.Identity,
                        bias=bt[:, m:m + 1])
                    nc.sync.dma_start(gates[blk, m], ot[:])

        # ================= PHASE 2: recurrent scan =================
        with tc.tile_pool(name="p2w", bufs=1) as p2w, \
             tc.tile_pool(name="p2g", bufs=2) as p2g, \
             tc.tile_pool(name="p2h", bufs=3) as p2h, \
             tc.tile_pool(name="p2t", bufs=3) as p2t, \
             tc.tile_pool(name="p2y", bufs=3) as p2y, \
             tc.tile_pool(name="p2ps", bufs=2, space="PSUM") as p2ps:
            wrz_t = p2w.tile([P, KT, 2 * H], BF16)
            nc.sync.dma_start(wrz_t[:], wrz[:])
            whh_t = p2w.tile([P, KT, H], BF16)
            nc.sync.dma_start(whh_t[:], whh[:])
            id_t = p2w.tile([P, P], BF16)
            nc.sync.dma_start(id_t[:], ident[:])

            hT = p2h.tile([P, KT, BL], F32, tag="hT")
            nc.sync.dma_start(hT[:], h0[:])

            for blk in range(nblk):
                gc = p2g.tile([P, MT_ALL, BL, TBLK], BF16, tag="gchunk")
                nc.sync.dma_start(
                    gc[:], gates[blk].rearrange("m p b t -> p m b t"))
                for tt in range(TBLK):
                    t = blk * TBLK + tt
                    if t >= t_steps:
                        break
                    # ---- h -> hi/lo bf16 ----
                    h_hi = p2t.tile([P, KT, BL], BF16, tag="h_hi")
                    nc.vector.tensor_copy(h_hi[:], hT[:])
                    h_rem = p2t.tile([P, KT, BL], F32, tag="h_rem")
                    nc.vector.tensor_sub(h_rem[:], hT[:], h_hi[:])
                    h_lo = p2t.tile([P, KT, BL], BF16, tag="h_lo")
                    nc.vector.tensor_copy(h_lo[:], h_rem[:])

                    # ---- r,z matmuls ----
                    ps_rz = p2ps.tile([P, MT_RZ, BL], F32, tag="ps_rz")
                    # gate add via identity matmul (one ldweights, 1 mm)
                    nc.tensor.matmul(
                        ps_rz[:].rearrange("p m b -> p (m b)"),
                        id_t[:],
                        gc[:, 0:MT_RZ, :, tt].rearrange("p m b -> p (m b)"),
                        start=True, stop=False, skip_group_check=True)
                    for m in range(MT_RZ):
                        for k in range(KT):
                            w_ap = wrz_t[:, k, m * P:(m + 1) * P]
                            nc.tensor.matmul(
                                ps_rz[:, m, :], w_ap, h_hi[:, k, :],
                                start=False, stop=False,
                                skip_group_check=True)
                            nc.tensor.matmul(
                                ps_rz[:, m, :], w_ap, h_lo[:, k, :],
                                start=False,
                                stop=(m == MT_RZ - 1 and k == KT - 1),
                                skip_group_check=True)
                    rz = p2t.tile([P, MT_RZ, BL], F32, tag="rz")
                    nc.scalar.activation(
                        rz[:], ps_rz[:], mybir.ActivationFunctionType.Sigmoid)

                    # ---- m = r * h (hi/lo) ----
                    mm_f = p2t.tile([P, KT, BL], F32, tag="mm_f")
                    nc.vector.tensor_mul(mm_f[:], rz[:, 0:KT, :], hT[:])
                    m_hi = p2t.tile([P, KT, BL], BF16, tag="m_hi")
                    nc.vector.tensor_copy(m_hi[:], mm_f[:])
                    m_rem = p2t.tile([P, KT, BL], F32, tag="m_rem")
                    nc.vector.tensor_sub(m_rem[:], mm_f[:], m_hi[:])
                    m_lo = p2t.tile([P, KT, BL], BF16, tag="m_lo")
                    nc.vector.tensor_copy(m_lo[:], m_rem[:])

                    # ---- candidate matmuls ----
                    ps_c = p2ps.tile([P, MT_H, BL], F32, tag="ps_c")
                    nc.tensor.matmul(
                        ps_c[:].rearrange("p m b -> p (m b)"),
                        id_t[:],
                        gc[:, MT_RZ:MT_ALL, :, tt].rearrange("p m b -> p (m b)"),
                        start=True, stop=False, skip_group_check=True)
                    for m in range(MT_H):
                        for k in range(KT):
                            w_ap = whh_t[:, k, m * P:(m + 1) * P]
                            nc.tensor.matmul(
                                ps_c[:, m, :], w_ap, m_hi[:, k, :],
                                start=False, stop=False,
                                skip_group_check=True)
                            nc.tensor.matmul(
                                ps_c[:, m, :], w_ap, m_lo[:, k, :],
                                start=False,
                                stop=(m == MT_H - 1 and k == KT - 1),
                                skip_group_check=True)
                    cT = p2t.tile([P, MT_H, BL], F32, tag="cT")
                    nc.scalar.activation(
                        cT[:], ps_c[:], mybir.ActivationFunctionType.Tanh)

                    # ---- blend: h' = c + z*(h - c) ----
                    z_ap = rz[:, KT:MT_RZ, :]
                    hmc = p2t.tile([P, KT, BL], F32, tag="hmc")
                    nc.vector.tensor_sub(hmc[:], hT[:], cT[:])
                    zh = p2t.tile([P, KT, BL], F32, tag="zh")
                    nc.vector.tensor_mul(zh[:], z_ap, hmc[:])
                    hT = p2h.tile([P, KT, BL], F32, tag="hT")
                    nc.vector.tensor_add(hT[:], cT[:], zh[:])

                    # ---- ys write (contiguous DMA, off critical path) ----
                    nc.sync.dma_start(ys[t], hT[:])

            nc.sync.dma_start(hT_out[:], hT[:])
    return nc


# ---------------------------------------------------------------- host side
def _prep_in_maps(x, state, w_xr_w, w_xr_b, w_hr_w, w_xz_w, w_xz_b, w_hz_w,
                  w_xh_w, w_xh_b, w_hh_w, t_steps=T):
    x = np.asarray(x)[:t_steps]
    state = np.asarray(state)

    # phase-1 stationary weights: w rows j -> lhsT [k, j]; [128, KT, 3H]
    wx_all = np.concatenate([w_xr_w, w_xz_w, w_xh_w], axis=0)  # [3H, I]
    wxT = np.ascontiguousarray(wx_all.T)                       # [I, 3H]
    wx_t = np.ascontiguousarray(
        wxT.reshape(KT, P, 3 * H).transpose(1, 0, 2)).astype(np.float32)

    bias_all = np.concatenate([w_xr_b, w_xz_b, w_xh_b])        # [3H]
    bias_t = np.ascontiguousarray(
        bias_all.reshape(MT_ALL, P).T).astype(np.float32)      # [128, 24]

    wrz_all = np.concatenate([w_hr_w, w_hz_w], axis=0)         # [2H, H]
    wrz_t = np.ascontiguousarray(
        wrz_all.T.reshape(KT, P, 2 * H).transpose(1, 0, 2)
    ).astype(ml_dtypes.bfloat16)
    whh_t = np.ascontiguousarray(
        w_hh_w.T.reshape(KT, P, H).transpose(1, 0, 2)
    ).astype(ml_dtypes.bfloat16)

    ident = np.eye(P, dtype=np.float32).astype(ml_dtypes.bfloat16)

    in_maps = []
    for c in range(NCORES):
        bs = slice(c * BL, (c + 1) * BL)
        xc = x[:, bs, :]                                       # [t, BL, I]
        # columns ordered (blk, b, t_within_blk)
        nblk = (t_steps + TBLK - 1) // TBLK
        xc_b = xc.reshape(nblk, TBLK, BL, I)
        # -> [I, blk, b, t] -> [P, KT, blk*b*t]
        xcT = np.ascontiguousarray(xc_b.transpose(3, 0, 2, 1))  # [I, blk, b, t]
        xT_c = np.ascontiguousarray(
            xcT.reshape(KT, P, nblk * BL * TBLK).transpose(1, 0, 2)
        ).astype(np.float32)

        h0_c = np.ascontiguousarray(
            state[bs, :].T.reshape(KT, P, BL).transpose(1, 0, 2)
        ).astype(np.float32)

        in_maps.append({
            "xT": xT_c, "wx": wx_t, "bias": bias_t,
            "wrz": wrz_t, "whh": whh_t, "ident": ident,
            "h0": h0_c,
        })
    return in_maps


def _assemble(results, t_steps=T):
    ys_parts = []
    hf_parts = []
    for c in range(NCORES):
        y = results[c]["ys"]                                   # [t, P, KT, BL]
        ys_parts.append(
            np.ascontiguousarray(y.transpose(0, 3, 2, 1)).reshape(t_steps, BL, H))
        hT_c = results[c]["hT_out"]                            # [P, KT, BL]
        hf_parts.append(hT_c.transpose(2, 1, 0).reshape(BL, H))
    ys = np.concatenate(ys_parts, axis=1)                      # [t, B, H]
    final = np.concatenate(hf_parts, axis=0)                   # [B, H]
    return ys, final


def kernel(x, state, w_xr_w, w_xr_b, w_hr_w, w_xz_w, w_xz_b, w_hz_w,
           w_xh_w, w_xh_b, w_hh_w, t_steps=T):
    _install_patch()
    key = t_steps
    if key not in _BUILD_CACHE:
        _BUILD_CACHE[key] = build(t_steps)
    nc = _BUILD_CACHE[key]
    in_maps = _prep_in_maps(x, state, w_xr_w, w_xr_b, w_hr_w, w_xz_w,
                            w_xz_b, w_hz_w, w_xh_w, w_xh_b, w_hh_w, t_steps)
    res = run_bass_kernel_spmd(nc, in_maps, core_ids=list(range(NCORES)))
    return _assemble(res.results, t_steps)


# revision 9
# speedup vs baseline: 123.3175x; 1.1870x over previous
"""GRU layer kernel for 8 Trainium2 NeuronCores.

Sharding: data-parallel over batch B=64 -> 8 cores x B_local=8 (no
cross-core communication; weights replicated, per-core batch slice).

Layout: everything transposed on-chip ([128 j-partitions, ...]):
 - Phase 1: gates g = x @ W_x.T + b precomputed for the whole sequence
   with fp32r matmuls (W stationary, xT streaming), biases folded into
   the PSUM->SBUF evict, stored bf16 in DRAM as [24 m][128 j][8 b][512 t].
 - Phase 2: 512-step scan. Recurrent matmuls W-stationary bf16 (FWL),
   h kept fp32 and split hi/lo bf16 per step for near-fp32 precision.
   Gate adds via identity matmul into the same PSUM accumulation.
"""
import sys
if '/opt/trn_rl_repo' not in sys.path:
    sys.path.insert(0, '/opt/trn_rl_repo')

import numpy as np
import ml_dtypes

import concourse.bass as bass
import concourse.mybir as mybir
import concourse.tile as tile
from concourse.bass_utils import run_bass_kernel_spmd

# ---------------------------------------------------------------- BIR patch
# This walrus build rejects >1 inline sync wait per instruction; hoist extras
# onto same-engine NoOps inserted immediately before (identical semantics).
import json as _json


def _split_multi_waits(bir_json: bytes) -> bytes:
    d = _json.loads(bir_json)
    uid = [0]

    def mk_nop(engine, wait, debug):
        uid[0] += 1
        return {
            "debug": debug, "engine": engine, "ins": [],
            "name": f"I-waitsplit-{uid[0]}", "opcode": "NoOp", "outs": [],
            "sync_info": {"on_wait": [wait], "on_update": []},
            "text_hint": "waitsplit",
        }

    for f in d["functions"]:
        for blk in f["blocks"]:
            out = []
            for inst in blk["instructions"]:
                si = inst.get("sync_info")
                ow = (si or {}).get("on_wait") or []
                if len(ow) > 1:
                    for w in ow[:-1]:
                        out.append(mk_nop(inst["engine"], w, inst.get("debug", 0)))
                    si["on_wait"] = [ow[-1]]
                out.append(inst)
            blk["instructions"] = out
    return _json.dumps(d).encode()


_PATCHED = False


def _install_patch():
    global _PATCHED
    if _PATCHED:
        return
    import concourse.bass_utils as bu
    import concourse.bass2jax as b2j
    orig = bu.compile_bir_kernel

    def patched(bir_json, tmpdir, neff_name="file.neff"):
        if isinstance(bir_json, str):
            bir_json = bir_json.encode()
        return orig(_split_multi_waits(bir_json), tmpdir, neff_name)

    bu.compile_bir_kernel = patched
    b2j.compile_bir_kernel = patched
    _PATCHED = True


# ---------------------------------------------------------------- constants
T, B, I, H = 512, 64, 1024, 1024
NCORES = 8
BL = B // NCORES            # 8 batch rows per core
P = 128
KT = I // P                 # 8 contraction tiles
MT_RZ = 2 * H // P          # 16 m-tiles for r,z
MT_H = H // P               # 8 m-tiles for candidate
MT_ALL = 3 * H // P         # 24 m-tiles in phase 1
TBLK = 64                   # phase-1 t-block / phase-2 gate chunk (steps)
NBLK = T // TBLK            # 8 blocks

F32 = mybir.dt.float32
F32R = mybir.dt.float32r
BF16 = mybir.dt.bfloat16

HMODE = "merged"   # "merged" (hi|lo lanes, 1 mm/tile) or "plain" (bf16 only)

_BUILD_CACHE = {}


def build(t_steps=T):
    nc = bass.Bass(trn_type="TRN2")
    nblk = (t_steps + TBLK - 1) // TBLK

    # ---- DRAM parameters (per-core shapes) ----
    # xT: [128, KT, BL*t_steps] fp32r  (partition=k within tile, k-tile, cols)
    # column order: (b, t) t-fastest within block? -> see host prep: (blk, b, t)
    xT = nc.dram_tensor("xT", [P, KT, BL * t_steps], F32R, kind="ExternalInput")
    # phase-1 weights (stationary): [128, KT, 3H] fp32r ; row j = m output
    wx = nc.dram_tensor("wx", [P, KT, 3 * H], F32R, kind="ExternalInput")
    # biases for the 24 m-tiles: [128, 24] fp32 (per-partition scalars)
    bias = nc.dram_tensor("bias", [P, MT_ALL], F32, kind="ExternalInput")
    # phase-2 weights bf16: [128, KT, 2H] and [128, KT, H]
    wrz = nc.dram_tensor("wrz", [P, KT, 2 * H], BF16, kind="ExternalInput")
    whh = nc.dram_tensor("whh", [P, KT, H], BF16, kind="ExternalInput")
    # identity 128x128 bf16
    ident = nc.dram_tensor("ident", [P, P], BF16, kind="ExternalInput")
    # initial state transposed: [128, KT, BL] fp32
    h0 = nc.dram_tensor("h0", [P, KT, BL], F32, kind="ExternalInput")
    # outputs
    # native layout [t, p, k, b]; host transposes to [t, b, H]
    ys = nc.dram_tensor("ys", [t_steps, P, KT, BL], F32, kind="ExternalOutput")
    hT_out = nc.dram_tensor("hT_out", [P, KT, BL], F32, kind="ExternalOutput")
    # intermediate gates in DRAM, bf16: [MT_ALL, 128, BL, t_steps]
    gates = nc.dram_tensor("gates", [nblk, MT_ALL, P, BL, TBLK], BF16)

    with tile.TileContext(nc) as tc:
        # ================= PHASE 1: gate projections =================
        with tc.tile_pool(name="p1w", bufs=1) as p1w, \
             tc.tile_pool(name="p1x", bufs=3) as p1x, \
             tc.tile_pool(name="p1o", bufs=4) as p1o, \
             tc.tile_pool(name="p1b", bufs=1) as p1b, \
             tc.tile_pool(name="p1ps", bufs=4, space="PSUM") as p1ps:
            # resident weights + biases
            wt = p1w.tile([P, KT, 3 * H], F32R)
            nc.sync.dma_start(wt[:], wx[:])
            bt = p1b.tile([P, MT_ALL], F32)
            nc.sync.dma_start(bt[:], bias[:])

            ncols = BL * TBLK  # 512 columns per t-block
            for blk in range(nblk):
                cols = slice(blk * ncols, (blk + 1) * ncols)
                xt_ = p1x.tile([P, KT, ncols], F32R, tag="xtile")
                nc.sync.dma_start(xt_[:], xT[:, :, cols])
                for m in range(MT_ALL):
                    ps = p1ps.tile([P, ncols], F32, tag="ps")
                    for k in range(KT):
                        nc.tensor.matmul(
                            ps[:], wt[:, k, m * P:(m + 1) * P], xt_[:, k, :],
                            start=(k == 0), stop=(k == KT - 1))
                    ot = p1o.tile([P, BL, TBLK], BF16, tag="gout")
                    # psum -> sbuf bf16 with bias folded in
                    nc.scalar.activation(
                        ot[:], ps[:].rearrange("p (b t) -> p b t", b=BL),
                        mybir.ActivationFunctionType.Identity,
                        bias=bt[:, m:m + 1])
                    nc.sync.dma_start(gates[blk, m], ot[:])

        # ================= PHASE 2: recurrent scan =================
        with tc.tile_pool(name="p2w", bufs=1) as p2w, \
             tc.tile_pool(name="p2g", bufs=2) as p2g, \
             tc.tile_pool(name="p2h", bufs=3) as p2h, \
             tc.tile_pool(name="p2t", bufs=3) as p2t, \
             tc.tile_pool(name="p2y", bufs=3) as p2y, \
             tc.tile_pool(name="p2ps", bufs=2, space="PSUM") as p2ps:
            wrz_t = p2w.tile([P, KT, 2 * H], BF16)
            nc.sync.dma_start(wrz_t[:], wrz[:])
            whh_t = p2w.tile([P, KT, H], BF16)
            nc.sync.dma_start(whh_t[:], whh[:])
            id_t = p2w.tile([P, P], BF16)
            nc.sync.dma_start(id_t[:], ident[:])

            hT = p2h.tile([P, KT, BL], F32, tag="hT")
            nc.sync.dma_start(hT[:], h0[:])

            for blk in range(nblk):
                gc = p2g.tile([P, MT_ALL, BL, TBLK], BF16, tag="gchunk")
                nc.sync.dma_start(
                    gc[:], gates[blk].rearrange("m p b t -> p m b t"))
                for tt in range(TBLK):
                    t = blk * TBLK + tt
                    if t >= t_steps:
                        break
                    lanes = 1 if HMODE == "plain" else 2

                    def mk_lanes(src_ap, tag):
                        """f32 -> [P, KT, lanes, BL] bf16 (hi | residual)."""
                        t2 = p2t.tile([P, KT, lanes, BL], BF16, tag=tag)
                        nc.vector.tensor_copy(t2[:, :, 0, :], src_ap)
                        if lanes == 2:
                            rem = p2t.tile([P, KT, BL], F32, tag=tag + "_r")
                            nc.vector.tensor_sub(rem[:], src_ap, t2[:, :, 0, :])
                            nc.vector.tensor_copy(t2[:, :, 1, :], rem[:])
                        return t2

                    def gate_group(w_tile, rhs2, g_ap, mt, tag):
                        """Accumulate identity*g + W.T @ rhs lanes; return
                        an f32 AP (SBUF or PSUM) of shape [P, mt, BL]."""
                        ps = p2ps.tile([P, mt, lanes, BL], F32, tag="ps_" + tag)
                        nc.tensor.matmul(
                            ps[:, :, 0, :], id_t[:], g_ap,
                            start=True, stop=False, skip_group_check=True)
                        for m in range(mt):
                            for k in range(KT):
                                nc.tensor.matmul(
                                    ps[:, m, :, :],
                                    w_tile[:, k, m * P:(m + 1) * P],
                                    rhs2[:, k, :, :],
                                    start=False,
                                    stop=(m == mt - 1 and k == KT - 1),
                                    skip_group_check=True)
                        if lanes == 1:
                            return ps[:, :, 0, :]
                        lo = p2t.tile([P, mt, BL], F32, tag="lo_" + tag)
                        nc.vector.tensor_copy(lo[:], ps[:, :, 1, :])
                        s = p2t.tile([P, mt, BL], F32, tag="s_" + tag)
                        nc.vector.tensor_add(s[:], ps[:, :, 0, :], lo[:])
                        return s[:]

                    # ---- r,z ----
                    h2 = mk_lanes(hT[:], "h2")
                    rz_pre = gate_group(
                        wrz_t, h2,
                        gc[:, 0:MT_RZ, :, tt].rearrange("p m b -> p (m b)"),
                        MT_RZ, "rz")
                    rz = p2t.tile([P, MT_RZ, BL], F32, tag="rz")
                    nc.scalar.activation(
                        rz[:], rz_pre, mybir.ActivationFunctionType.Sigmoid)

                    # ---- m = r * h ----
                    mm_f = p2t.tile([P, KT, BL], F32, tag="mm_f")
                    nc.vector.tensor_mul(mm_f[:], rz[:, 0:KT, :], hT[:])
                    m2 = mk_lanes(mm_f[:], "m2")

                    # ---- candidate ----
                    c_pre = gate_group(
                        whh_t, m2,
                        gc[:, MT_RZ:MT_ALL, :, tt].rearrange("p m b -> p (m b)"),
                        MT_H, "c")
                    cT = p2t.tile([P, MT_H, BL], F32, tag="cT")
                    nc.scalar.activation(
                        cT[:], c_pre, mybir.ActivationFunctionType.Tanh)

                    # ---- blend: h' = c + z*(h - c) ----
                    z_ap = rz[:, KT:MT_RZ, :]
                    hmc = p2t.tile([P, KT, BL], F32, tag="hmc")
                    nc.vector.tensor_sub(hmc[:], hT[:], cT[:])
                    zh = p2t.tile([P, KT, BL], F32, tag="zh")
                    nc.vector.tensor_mul(zh[:], z_ap, hmc[:])
                    hT = p2h.tile([P, KT, BL], F32, tag="hT")
                    nc.vector.tensor_add(hT[:], cT[:], zh[:])

                    # ---- ys write (contiguous DMA, off critical path) ----
                    nc.sync.dma_start(ys[t], hT[:])

            nc.sync.dma_start(hT_out[:], hT[:])
    return nc


# ---------------------------------------------------------------- host side
def _prep_in_maps(x, state, w_xr_w, w_xr_b, w_hr_w, w_xz_w, w_xz_b, w_hz_w,
                  w_xh_w, w_xh_b, w_hh_w, t_steps=T):
    x = np.asarray(x)[:t_steps]
    state = np.asarray(state)

    # phase-1 stationary weights: w rows j -> lhsT [k, j]; [128, KT, 3H]
    wx_all = np.concatenate([w_xr_w, w_xz_w, w_xh_w], axis=0)  # [3H, I]
    wxT = np.ascontiguousarray(wx_all.T)                       # [I, 3H]
    wx_t = np.ascontiguousarray(
        wxT.reshape(KT, P, 3 * H).transpose(1, 0, 2)).astype(np.float32)

    bias_all = np.concatenate([w_xr_b, w_xz_b, w_xh_b])        # [3H]
    bias_t = np.ascontiguousarray(
        bias_all.reshape(MT_ALL, P).T).astype(np.float32)      # [128, 24]

    wrz_all = np.concatenate([w_hr_w, w_hz_w], axis=0)         # [2H, H]
    wrz_t = np.ascontiguousarray(
        wrz_all.T.reshape(KT, P, 2 * H).transpose(1, 0, 2)
    ).astype(ml_dtypes.bfloat16)
    whh_t = np.ascontiguousarray(
        w_hh_w.T.reshape(KT, P, H).transpose(1, 0, 2)
    ).astype(ml_dtypes.bfloat16)

    ident = np.eye(P, dtype=np.float32).astype(ml_dtypes.bfloat16)

    in_maps = []
    for c in range(NCORES):
        bs = slice(c * BL, (c + 1) * BL)
        xc = x[:, bs, :]                                       # [t, BL, I]
        # columns ordered (blk, b, t_within_blk)
        nblk = (t_steps + TBLK - 1) // TBLK
        xc_b = xc.reshape(nblk, TBLK, BL, I)
        # -> [I, blk, b, t] -> [P, KT, blk*b*t]
        xcT = np.ascontiguousarray(xc_b.transpose(3, 0, 2, 1))  # [I, blk, b, t]
        xT_c = np.ascontiguousarray(
            xcT.reshape(KT, P, nblk * BL * TBLK).transpose(1, 0, 2)
        ).astype(np.float32)

        h0_c = np.ascontiguousarray(
            state[bs, :].T.reshape(KT, P, BL).transpose(1, 0, 2)
        ).astype(np.float32)

        in_maps.append({
            "xT": xT_c, "wx": wx_t, "bias": bias_t,
            "wrz": wrz_t, "whh": whh_t, "ident": ident,
            "h0": h0_c,
        })
    return in_maps


def _assemble(results, t_steps=T):
    ys_parts = []
    hf_parts = []
    for c in range(NCORES):
        y = results[c]["ys"]                                   # [t, P, KT, BL]
        ys_parts.append(
            np.ascontiguousarray(y.transpose(0, 3, 2, 1)).reshape(t_steps, BL, H))
        hT_c = results[c]["hT_out"]                            # [P, KT, BL]
        hf_parts.append(hT_c.transpose(2, 1, 0).reshape(BL, H))
    ys = np.concatenate(ys_parts, axis=1)                      # [t, B, H]
    final = np.concatenate(hf_parts, axis=0)                   # [B, H]
    return ys, final


def kernel(x, state, w_xr_w, w_xr_b, w_hr_w, w_xz_w, w_xz_b, w_hz_w,
           w_xh_w, w_xh_b, w_hh_w, t_steps=T):
    _install_patch()
    key = t_steps
    if key not in _BUILD_CACHE:
        _BUILD_CACHE[key] = build(t_steps)
    nc = _BUILD_CACHE[key]
    in_maps = _prep_in_maps(x, state, w_xr_w, w_xr_b, w_hr_w, w_xz_w,
                            w_xz_b, w_hz_w, w_xh_w, w_xh_b, w_hh_w, t_steps)
    res = run_bass_kernel_spmd(nc, in_maps, core_ids=list(range(NCORES)))
    return _assemble(res.results, t_steps)
